# revision 8
# baseline (speedup 1.0000x reference)
"""Mixture-of-logistics NLL loss (reduction=mean) on 8 Trainium2 NeuronCores.

Math (per row, K=16 mixture components):
    log_prob = logsumexp_k(logw_k + comp_k) where logw = log_softmax(w)
             = log(sum_k e^{w_k} * pdf_k) - log(sum_k e^{w_k})
    pdf_k = logistic_pdf(t; loc_k, s_k) = (1 - tanh^2(z_k/2)) / (4 s_k),
            z_k = (t - loc_k)/s_k
Using rp = 1/s = exp(-ln(s)):
    pdf = (1 - th^2)/4 * rp,  th = tanh(0.5 * (t - loc) * rp)
Output = mean over all rows of log_prob.

Sharding: pure data parallel over rows (batch*seq) across 8 cores; each core
returns per-partition partial sums [128, 2] = (sum ln(num), sum ln(den));
host combines.

ACT table-set phasing (avoids ~2.7us table reloads per switch):
  phase A: Ln(scale) only            (natural_log_exp set)
  phase B: Exp, Tanh                 (exp_and_others set)
  phase C: Ln of row-sums + accum    (natural_log_exp set)
Cross-phase ACT ordering is pinned with scheduler-only deps.
"""

import numpy as np

import concourse.bass as bass
import concourse.bacc as bacc
import concourse.mybir as mybir
import concourse.tile as tile
from concourse.tile_rust import add_dep_helper
from concourse.bass_utils import run_bass_kernel_spmd

B, T, K = 16, 131072, 16
N = B * T                 # 2097152 rows total
NCORES = 8
NLOC = N // NCORES        # 262144 rows per core
P = 128                   # SBUF partitions

F32 = mybir.dt.float32
BF16 = mybir.dt.bfloat16
AF = mybir.ActivationFunctionType
OP = mybir.AluOpType
LN2 = float(np.log(2.0))


def build_kernel(nloc=NLOC, c=256):
    """Build the per-core Bass module. nloc rows, c rows per partition-tile."""
    p = P
    r = nloc // p             # rows per partition
    nt = r // c               # tiles
    assert r % c == 0 and nloc % p == 0

    nc = bacc.Bacc("TRN2", target_bir_lowering=False, debug=False)
    w_d = nc.dram_tensor("w", [nloc, K], F32, kind="ExternalInput")
    loc_d = nc.dram_tensor("loc", [nloc, K], F32, kind="ExternalInput")
    scale_d = nc.dram_tensor("scale", [nloc, K], F32, kind="ExternalInput")
    t_d = nc.dram_tensor("t", [nloc], F32, kind="ExternalInput")
    out_d = nc.dram_tensor("out", [p, 2], F32, kind="ExternalOutput")

    wv = w_d.ap().rearrange("(p r) k -> p r k", p=p)
    lv = loc_d.ap().rearrange("(p r) k -> p r k", p=p)
    sv = scale_d.ap().rearrange("(p r) k -> p r k", p=p)
    tv = t_d.ap().rearrange("(p r) -> p r", p=p)

    with tile.TileContext(nc) as tc:
        with tc.tile_pool(name="persist", bufs=1) as pp:
            u_all = pp.tile([p, r, K], BF16)      # ln(scale), bf16
            t_all = pp.tile([p, r], F32)          # targets
            stash_s = pp.tile([p, r], F32)        # per-row numerator sums
            stash_w = pp.tile([p, r], F32)        # per-row denominator sums
            out_sb = pp.tile([p, 2], F32)

            nc.gpsimd.dma_start(out=t_all, in_=tv)

            # ---- phase A: u = ln(scale) (ACT set: natural_log_exp) ----
            a_acts = []
            with tc.tile_pool(name="pa", bufs=2) as pa:
                for i in range(nt):
                    sl = slice(i * c, (i + 1) * c)
                    sc_t = pa.tile([p, c, K], F32, tag="sc")
                    nc.gpsimd.dma_start(out=sc_t, in_=sv[:, sl, :])
                    ia = nc.scalar.activation(
                        out=u_all[:, sl, :], in_=sc_t, func=AF.Ln
                    )
                    a_acts.append(ia)

            # ---- phase B: everything except final logs (ACT set: exp) ----
            b_acts = []
            with (
                tc.tile_pool(name="pb", bufs=2) as pb,
                tc.tile_pool(name="pt", bufs=2) as pt,
                nc.allow_low_precision("bf16 partial sums validated: 3.5e-4 rel"),
            ):
                for i in range(nt):
                    sl = slice(i * c, (i + 1) * c)
                    loc_t = pb.tile([p, c, K], BF16, tag="loc")
                    w_t = pb.tile([p, c, K], BF16, tag="w")
                    # SWDGE DMAs cast f32->bf16 in flight
                    nc.gpsimd.dma_start(out=loc_t, in_=lv[:, sl, :])
                    nc.gpsimd.dma_start(out=w_t, in_=wv[:, sl, :])

                    rp_t = pb.tile([p, c, K], BF16, tag="rp")
                    b_acts.append(
                        nc.scalar.activation(
                            out=rp_t, in_=u_all[:, sl, :], func=AF.Exp,
                            scale=-1.0,
                        )
                    )
                    ew_t = pb.tile([p, c, K], BF16, tag="ew")
                    b_acts.append(
                        nc.scalar.activation(out=ew_t, in_=w_t, func=AF.Exp)
                    )

                    ch = pb.tile([p, c, K], BF16, tag="ch")
                    tb = t_all[:, sl].unsqueeze(2).broadcast_to([p, c, K])
                    nc.vector.tensor_sub(out=ch, in0=tb, in1=loc_t)
                    nc.vector.tensor_mul(out=ch, in0=ch, in1=rp_t)
                    b_acts.append(
                        nc.scalar.activation(out=ch, in_=ch, func=AF.Tanh, scale=0.5)
                    )
                    nc.vector.tensor_mul(out=ch, in0=ch, in1=ch)          # th^2
                    nc.vector.tensor_scalar(
                        out=ch, in0=ch, scalar1=-0.25, scalar2=0.25,
                        op0=OP.mult, op1=OP.add,
                    )                                                     # (1-th^2)/4
                    nc.vector.tensor_mul(out=ch, in0=ch, in1=rp_t)        # * rp
                    nc.vector.tensor_mul(out=ch, in0=ch, in1=ew_t)        # * e^w

                    # tree-reduce term over K -> stash_s[:, sl]
                    t1 = pt.tile([p, c, 8], BF16, tag="t1")
                    nc.vector.tensor_add(out=t1, in0=ch[:, :, 0:8], in1=ch[:, :, 8:16])
                    t2 = pt.tile([p, c, 4], BF16, tag="t2")
                    nc.vector.tensor_add(out=t2, in0=t1[:, :, 0:4], in1=t1[:, :, 4:8])
                    t3 = pt.tile([p, c, 2], BF16, tag="t3")
                    nc.vector.tensor_add(out=t3, in0=t2[:, :, 0:2], in1=t2[:, :, 2:4])
                    nc.vector.tensor_add(
                        out=stash_s[:, sl], in0=t3[:, :, 0], in1=t3[:, :, 1]
                    )
                    # tree-reduce e^w over K -> stash_w[:, sl]
                    e1 = pt.tile([p, c, 8], BF16, tag="t1")
                    nc.vector.tensor_add(out=e1, in0=ew_t[:, :, 0:8], in1=ew_t[:, :, 8:16])
                    e2 = pt.tile([p, c, 4], BF16, tag="t2")
                    nc.vector.tensor_add(out=e2, in0=e1[:, :, 0:4], in1=e1[:, :, 4:8])
                    e3 = pt.tile([p, c, 2], BF16, tag="t3")
                    nc.vector.tensor_add(out=e3, in0=e2[:, :, 0:2], in1=e2[:, :, 2:4])
                    nc.vector.tensor_add(
                        out=stash_w[:, sl], in0=e3[:, :, 0], in1=e3[:, :, 1]
                    )

            # ---- phase C: per-row logs + per-partition accumulation ----
            c_acts = [
                nc.scalar.activation(
                    out=stash_s, in_=stash_s, func=AF.Ln,
                    accum_out=out_sb[:, 0:1],
                ),
                nc.scalar.activation(
                    out=stash_w, in_=stash_w, func=AF.Ln,
                    accum_out=out_sb[:, 1:2],
                ),
            ]
            nc.gpsimd.dma_start(out=out_d.ap(), in_=out_sb)

            # Pin ACT phase order (scheduler-only edges; same engine, no sems)
            for b in b_acts:
                for a in a_acts:
                    add_dep_helper(b.ins, a.ins, False, "act-table-phase-AB")
            for cact in c_acts:
                for b in b_acts:
                    add_dep_helper(cact.ins, b.ins, False, "act-table-phase-BC")

    nc.compile()
    return nc


def _combine(outs, n_rows):
    total = 0.0
    for o in outs:
        total += float(o[:, 0].sum(dtype=np.float64))
        total -= float(o[:, 1].sum(dtype=np.float64))
    return np.float32(total / n_rows)


def make_in_maps(weight, loc, scale, targets):
    w = np.ascontiguousarray(weight.reshape(N, K), dtype=np.float32)
    l = np.ascontiguousarray(loc.reshape(N, K), dtype=np.float32)
    s = np.ascontiguousarray(scale.reshape(N, K), dtype=np.float32)
    t = np.ascontiguousarray(targets.reshape(N), dtype=np.float32)
    in_maps = []
    for ci in range(NCORES):
        rs = slice(ci * NLOC, (ci + 1) * NLOC)
        in_maps.append({
            "w": np.ascontiguousarray(w[rs]),
            "loc": np.ascontiguousarray(l[rs]),
            "scale": np.ascontiguousarray(s[rs]),
            "t": np.ascontiguousarray(t[rs]),
        })
    return in_maps


def run(in_maps, **kwargs):
    nc = build_kernel()
    return run_bass_kernel_spmd(nc, in_maps, core_ids=list(range(NCORES)), **kwargs)


def kernel(weight, loc, scale, targets):
    res = run(make_in_maps(weight, loc, scale, targets))
    return _combine([r["out"] for r in res.results], N)


if __name__ == "__main__":
    nc = build_kernel()
    print("kernel built OK")


# revision 9
# speedup vs baseline: 1.1083x; 1.1083x over previous
"""Mixture-of-logistics NLL loss (reduction=mean) on 8 Trainium2 NeuronCores.

Math (per row, K=16 mixture components):
    log_prob = logsumexp_k(logw_k + comp_k) where logw = log_softmax(w)
             = log(sum_k e^{w_k} * pdf_k) - log(sum_k e^{w_k})
    pdf_k = logistic_pdf(t; loc_k, s_k) = (1 - tanh^2(z_k/2)) / (4 s_k),
            z_k = (t - loc_k)/s_k
Using rp = 1/s = exp(-ln(s)):
    pdf = (1 - th^2)/4 * rp,  th = tanh(0.5 * (t - loc) * rp)
    term = e^w * pdf = ((1-th^2)/4) * (rp * e^w)
Output = mean over all rows of log_prob.

Sharding: pure data parallel over rows (batch*seq) across 8 cores; each core
returns per-partition partial sums [128, 2] = (sum ln(num), sum ln(den));
host combines.

ACT table-set discipline (a set switch costs ~1.3us table DMA):
  phase A (per chunk): Ln(scale), Exp(-u), Exp(w)   (natural_log_exp set)
  phase B (per chunk): Tanh                          (exp_and_others set)
  phase C (end): Ln of row-sums + accumulate         (natural_log_exp set)
Tiles are processed in chunks of 2; A/B alternate per chunk so all engines
stay busy while only ~2 tiles of cross-phase state (v, rp*e^w) stay live.
All ACT ops are chained with scheduler-only deps to pin this order.

The broadcast subtract (t - loc) runs on GpSimd to keep DVE (the busiest
engine) off the 1x-mode broadcast op.
"""

import numpy as np

import concourse.bacc as bacc
import concourse.mybir as mybir
import concourse.tile as tile
from concourse.tile_rust import add_dep_helper
from concourse.bass_utils import run_bass_kernel_spmd

B, T, K = 16, 131072, 16
N = B * T                 # 2097152 rows total
NCORES = 8
NLOC = N // NCORES        # 262144 rows per core
P = 128                   # SBUF partitions

F32 = mybir.dt.float32
BF16 = mybir.dt.bfloat16
AF = mybir.ActivationFunctionType
OP = mybir.AluOpType


def build_kernel(nloc=NLOC, c=256, chunk=2, diff_on_pool=True):
    """Build the per-core Bass module. nloc rows, c rows per partition-tile."""
    p = P
    r = nloc // p             # rows per partition
    nt = r // c               # tiles
    assert r % c == 0 and nloc % p == 0 and nt % chunk == 0

    nc = bacc.Bacc("TRN2", target_bir_lowering=False, debug=False)
    w_d = nc.dram_tensor("w", [nloc, K], F32, kind="ExternalInput")
    loc_d = nc.dram_tensor("loc", [nloc, K], F32, kind="ExternalInput")
    scale_d = nc.dram_tensor("scale", [nloc, K], F32, kind="ExternalInput")
    t_d = nc.dram_tensor("t", [nloc], F32, kind="ExternalInput")
    out_d = nc.dram_tensor("out", [p, 2], F32, kind="ExternalOutput")

    wv = w_d.ap().rearrange("(p r) k -> p r k", p=p)
    lv = loc_d.ap().rearrange("(p r) k -> p r k", p=p)
    sv = scale_d.ap().rearrange("(p r) k -> p r k", p=p)
    tv = t_d.ap().rearrange("(p r) -> p r", p=p)

    acts = []  # every ACT instruction, in required execution order

    def act(*args, **kwargs):
        ins = nc.scalar.activation(*args, **kwargs)
        acts.append(ins)
        return ins

    with tile.TileContext(nc) as tc:
        with (
            tc.tile_pool(name="persist", bufs=1) as pp,
            tc.tile_pool(name="psc", bufs=2) as psc,
            tc.tile_pool(name="pwld", bufs=3) as pwld,
            tc.tile_pool(name="plc", bufs=3) as plc,
            tc.tile_pool(name="prp", bufs=2) as prp,
            tc.tile_pool(name="pv", bufs=2 * chunk) as pv,
            tc.tile_pool(name="ppw", bufs=2 * chunk) as ppw,
            tc.tile_pool(name="pt", bufs=2) as pt,
            nc.allow_low_precision("bf16 partial sums validated: 3.5e-4 rel"),
        ):
            t_all = pp.tile([p, r], F32)          # targets
            stash_s = pp.tile([p, r], F32)        # per-row numerator sums
            stash_w = pp.tile([p, r], F32)        # per-row denominator sums
            out_sb = pp.tile([p, 2], F32)

            nc.gpsimd.dma_start(out=t_all, in_=tv)

            def tree16(src, dst_slice):
                """Sum src [p, c, 16] bf16 over last axis -> dst_slice [p, c] f32."""
                t1 = pt.tile([p, c, 8], BF16, tag="t1")
                nc.vector.tensor_add(out=t1, in0=src[:, :, 0:8], in1=src[:, :, 8:16])
                t2 = pt.tile([p, c, 4], BF16, tag="t2")
                nc.vector.tensor_add(out=t2, in0=t1[:, :, 0:4], in1=t1[:, :, 4:8])
                t3 = pt.tile([p, c, 2], BF16, tag="t3")
                nc.vector.tensor_add(out=t3, in0=t2[:, :, 0:2], in1=t2[:, :, 2:4])
                nc.vector.tensor_add(out=dst_slice, in0=t3[:, :, 0], in1=t3[:, :, 1])

            for h in range(nt // chunk):
                tiles = range(h * chunk, (h + 1) * chunk)
                # ---- phase A of chunk: Ln + both Exps + v/pw/den-tree ----
                vts, pwts = {}, {}
                for i in tiles:
                    sl = slice(i * c, (i + 1) * c)
                    sc_t = psc.tile([p, c, K], BF16, tag="sc")
                    w_t = pwld.tile([p, c, K], BF16, tag="w")
                    loc_t = plc.tile([p, c, K], BF16, tag="loc")
                    # SWDGE DMAs cast f32->bf16 in flight
                    nc.gpsimd.dma_start(out=sc_t, in_=sv[:, sl, :])
                    nc.gpsimd.dma_start(out=w_t, in_=wv[:, sl, :])
                    nc.gpsimd.dma_start(out=loc_t, in_=lv[:, sl, :])

                    act(out=sc_t, in_=sc_t, func=AF.Ln)          # u, in place
                    rp_t = prp.tile([p, c, K], BF16, tag="rp")
                    act(out=rp_t, in_=sc_t, func=AF.Exp, scale=-1.0)   # 1/s
                    act(out=w_t, in_=w_t, func=AF.Exp)           # e^w, in place

                    # diff = t - loc (broadcast over K), in place over loc
                    tb = t_all[:, sl].unsqueeze(2).broadcast_to([p, c, K])
                    eng = nc.gpsimd if diff_on_pool else nc.vector
                    eng.tensor_sub(out=loc_t, in0=tb, in1=loc_t)

                    v_t = pv.tile([p, c, K], BF16, tag="v")
                    nc.vector.tensor_mul(out=v_t, in0=loc_t, in1=rp_t)
                    pw_t = ppw.tile([p, c, K], BF16, tag="pw")
                    nc.vector.tensor_mul(out=pw_t, in0=rp_t, in1=w_t)
                    tree16(w_t, stash_w[:, sl])                  # sum e^w
                    vts[i], pwts[i] = v_t, pw_t

                # ---- phase B of chunk: tanh + q/term + num-tree ----
                for i in tiles:
                    sl = slice(i * c, (i + 1) * c)
                    v_t, pw_t = vts[i], pwts[i]
                    act(out=v_t, in_=v_t, func=AF.Tanh, scale=0.5)     # th
                    nc.vector.tensor_mul(out=v_t, in0=v_t, in1=v_t)    # th^2
                    nc.vector.tensor_scalar(
                        out=v_t, in0=v_t, scalar1=-0.25, scalar2=0.25,
                        op0=OP.mult, op1=OP.add,
                    )                                                  # (1-th^2)/4
                    nc.vector.tensor_mul(out=v_t, in0=v_t, in1=pw_t)   # term
                    tree16(v_t, stash_s[:, sl])

            # ---- phase C: per-row logs + per-partition accumulation ----
            act(out=stash_s, in_=stash_s, func=AF.Ln, accum_out=out_sb[:, 0:1])
            act(out=stash_w, in_=stash_w, func=AF.Ln, accum_out=out_sb[:, 1:2])
            nc.gpsimd.dma_start(out=out_d.ap(), in_=out_sb)

            # Pin ACT execution order (same engine -> scheduler-only edges)
            for prev, nxt in zip(acts, acts[1:]):
                add_dep_helper(nxt.ins, prev.ins, False, "act-table-order")

    nc.compile()
    return nc


def _combine(outs, n_rows):
    total = 0.0
    for o in outs:
        total += float(o[:, 0].sum(dtype=np.float64))
        total -= float(o[:, 1].sum(dtype=np.float64))
    return np.float32(total / n_rows)


def make_in_maps(weight, loc, scale, targets):
    w = np.ascontiguousarray(weight.reshape(N, K), dtype=np.float32)
    l = np.ascontiguousarray(loc.reshape(N, K), dtype=np.float32)
    s = np.ascontiguousarray(scale.reshape(N, K), dtype=np.float32)
    t = np.ascontiguousarray(targets.reshape(N), dtype=np.float32)
    in_maps = []
    for ci in range(NCORES):
        rs = slice(ci * NLOC, (ci + 1) * NLOC)
        in_maps.append({
            "w": np.ascontiguousarray(w[rs]),
            "loc": np.ascontiguousarray(l[rs]),
            "scale": np.ascontiguousarray(s[rs]),
            "t": np.ascontiguousarray(t[rs]),
        })
    return in_maps


def run(in_maps, **kwargs):
    nc = build_kernel()
    return run_bass_kernel_spmd(nc, in_maps, core_ids=list(range(NCORES)), **kwargs)


def kernel(weight, loc, scale, targets):
    res = run(make_in_maps(weight, loc, scale, targets))
    return _combine([r["out"] for r in res.results], N)


if __name__ == "__main__":
    nc = build_kernel()
    print("kernel built OK")
